# revision 37
# baseline (speedup 1.0000x reference)
"""Trainium2 Bass kernel for nn_ConvPersonGraphical (GNN message passing block).

Reference computation (per batch element n):
    h   = conv_w @ x[n]^T + conv_b          # 1x1 conv -> (2*OUT, V)
    h   = BN_train(h)                        # batch stats over (N, V) per channel
    hk  = h.reshape(K, OUT, V)
    t_k = h_k @ A[n,k]                       # (OUT, V) @ (V, V)
    y   = sum_k t_k @ A[n,k]^T               # (OUT, V)
    out = (1 + eps_p) * x[n] + y^T
    also returns At = transpose(A, (0,1,3,2))

Strategy: data-parallel over batch N across 8 NeuronCores (8 per core).
The conv and both propagation matmuls run at full TensorE rate in bf16
(numerically safe: the propagation output is tiny relative to the
residual x, which is carried in exact fp32).  BatchNorm batch stats:
S1 factors through the conv (S1 = colsum(x) @ w^T, almost free); S2 is
accumulated with ones-vector matmuls on the squared activations.  The
global reduction is one small AllReduce (preceded by a dummy warm-up
collective that absorbs the ncfw init cost under phase A).  A^T is
produced exactly on the TensorEngine in fp32 (needed as the stationary
operand of the second propagation matmul anyway) and streamed out as
the second output.  hT spills to DRAM between the phases to keep SBUF
free for deep pipelining.
"""

import os
import sys
import types

import numpy as np

import concourse.bass as bass
import concourse.bacc as bacc
import concourse.mybir as mybir
import concourse.tile as tile
from concourse.alu_op_type import AluOpType
from concourse.bass_utils import run_bass_kernel_spmd


def _install_ntff_hook_shim():
    """concourse.bass_utils reads antenv.axon_hooks.get_axon_ntff_profile_hook
    for trace=True under axon; this image's antenv lacks that module, so
    register an equivalent backed by the booted libaxon_pjrt.so."""
    try:
        import antenv.axon_hooks  # noqa: F401
        return
    except ImportError:
        pass
    try:
        import antenv
        from trn_agent_boot.trn_boot import _ntff_profile_via_ctypes

        hook = _ntff_profile_via_ctypes("/opt/axon/libaxon_pjrt.so")
        mod = types.ModuleType("antenv.axon_hooks")
        mod._hook = hook
        mod.get_axon_ntff_profile_hook = lambda: mod._hook

        def _set(h):
            mod._hook = h

        mod.set_axon_ntff_profile_hook = _set
        sys.modules["antenv.axon_hooks"] = mod
        antenv.axon_hooks = mod
    except Exception:
        pass


_install_ntff_hook_shim()

def _install_ldw_opt_patch():
    """Re-enable walrus LDWEIGHTS scheduling opt (hidden weight loads)."""
    if os.environ.get("KLDW", "0") != "1":
        return
    import concourse.bass_utils as _bu
    if getattr(_bu, "_ldw_patched", False):
        return
    _orig = _bu.run_command

    def _run(cmd, *a, **kw):
        if isinstance(cmd, list):
            cmd = ["--enable-ldw-opt=true" if c == "--enable-ldw-opt=false"
                   else c for c in cmd]
        return _orig(cmd, *a, **kw)

    _bu.run_command = _run
    _bu._ldw_patched = True


_install_ldw_opt_patch()

F32 = mybir.dt.float32
F32R = mybir.dt.float32r
BF16 = mybir.dt.bfloat16
AF = mybir.ActivationFunctionType

NCORES = 8
N, V, C = 64, 512, 512
K = 2
OUT2 = 1024            # 2 * OUT
NS = N // NCORES       # batch elements per core
COUNT = N * V          # BN statistic count (global, over all cores)
BN_EPS = 1e-5

P = 128                # SBUF partitions
VT = V // P            # 4 tiles of 128 along any 512 dim
OH = OUT2 // 512       # 2 halves of the channel dim (PSUM bank = 512 fp32)

PREF = 4               # batch elements whose A-path runs under the collective


def build(nc: bacc.Bacc):
    x_d = nc.declare_dram_parameter("x", [NS, V, C], F32, isOutput=False)
    xT_d = nc.declare_dram_parameter("xTb", [NS, C, V], BF16, isOutput=False)
    A_d = nc.declare_dram_parameter("A", [NS, K, V, V], F32, isOutput=False)
    w_d = nc.declare_dram_parameter("conv_w", [OUT2, C], F32, isOutput=False)
    nc.declare_dram_parameter("conv_b", [OUT2], F32, isOutput=False)
    g_d = nc.declare_dram_parameter("gamma", [OUT2], F32, isOutput=False)
    be_d = nc.declare_dram_parameter("beta", [OUT2], F32, isOutput=False)
    ep_d = nc.declare_dram_parameter("eps_p", [1], F32, isOutput=False)
    id_d = nc.declare_dram_parameter("identity", [P, P], F32, isOutput=False)
    out_d = nc.declare_dram_parameter("out", [NS, V, C], F32, isOutput=True)
    At_d = nc.declare_dram_parameter("At", [NS, K, V, V], F32, isOutput=True)

    with tile.TileContext(nc) as tc:
        _graph(nc, tc, x_d, xT_d, A_d, w_d, g_d, be_d, ep_d, id_d, out_d, At_d)
    nc.compile()
    return nc


def _graph(nc, tc, x_d, xT_d, A_d, w_d, g_d, be_d, ep_d, id_d, out_d, At_d):
    with (
        tc.tile_pool(name="const", bufs=1) as constp,
        tc.tile_pool(name="rows", bufs=1) as rowsp,
        tc.tile_pool(name="dram", bufs=1, space="DRAM") as dramp,
    ):
        # ---------------- constants ----------------
        identity = constp.tile([P, P], F32)
        nc.sync.dma_start(identity[:], id_d.ap())
        identity_bf = constp.tile([P, P], BF16)
        nc.vector.tensor_copy(identity_bf[:], identity[:])
        ones_col = constp.tile([P, 1], BF16)
        nc.vector.memset(ones_col[:], 1.0)
        ones_row_f = rowsp.tile([1, P], F32)
        nc.vector.memset(ones_row_f[:], 1.0)
        ones_row = rowsp.tile([1, P], F32R)
        nc.vector.tensor_copy(ones_row[:], ones_row_f[:])
        g_row = rowsp.tile([1, OUT2], F32)
        nc.sync.dma_start(g_row[:], g_d.ap().unsqueeze(0))
        be_row = rowsp.tile([1, OUT2], F32)
        nc.sync.dma_start(be_row[:], be_d.ap().unsqueeze(0))
        ep_sb = rowsp.tile([1, 1], F32)
        nc.sync.dma_start(ep_sb[:], ep_d.ap().unsqueeze(0))
        xsum = rowsp.tile([P, VT], F32)
        nc.vector.memset(xsum[:], 0.0)
        xred = rowsp.tile([P, 1], F32)

        # dummy collective issued first (no deps): absorbs ncfw init cost
        # under phase A so the real stats AllReduce hits the warm path
        dummy_sb = rowsp.tile([1, 32], F32)
        nc.vector.memset(dummy_sb[:], 0.0)
        dummy_in = dramp.tile([1, 32], F32)
        dummy_out = dramp.tile([NCORES, 32], F32)
        nc.sync.dma_start(dummy_in[:], dummy_sb[:])
        nc.gpsimd.collective_compute(
            "AllGather",
            AluOpType.bypass,
            replica_groups=[list(range(NCORES))],
            ins=[dummy_in.opt()],
            outs=[dummy_out.opt()],
        )
        ones8 = rowsp.tile([NCORES, 1], F32)
        nc.vector.memset(ones8[:], 1.0)

        # hT spill buffer in DRAM (bf16): [n][p][vt][o]
        hT_dram = dramp.tile([NS, P, VT, OUT2], BF16)

        # ---------------- phase A: conv + BN statistics ----------------
        with (
            tc.tile_pool(name="wload", bufs=1) as wl,
            tc.tile_pool(name="wT", bufs=1) as wTp,
            tc.tile_pool(name="xT", bufs=3) as xTp,
            tc.tile_pool(name="hTa", bufs=3) as hTap,
            tc.tile_pool(name="sq", bufs=4) as sqp,
            tc.tile_pool(name="psA", bufs=1, space="PSUM") as psA,   # transposes
            tc.tile_pool(name="psC", bufs=3, space="PSUM") as psC,   # conv
            tc.tile_pool(name="psS", bufs=1, space="PSUM") as psS,   # stats
        ):
            # transpose conv_w (OUT2, C) -> wT (C, OUT2), in bf16
            w_nat = wl.tile([P, OUT2 // P, C], F32)
            nc.sync.dma_start(
                w_nat[:], w_d.ap().rearrange("(oo p) c -> p oo c", p=P)
            )
            w_natb = wl.tile([P, OUT2 // P, C], BF16)
            for oo in range(OUT2 // P):
                nc.vector.tensor_copy(w_natb[:, oo, :], w_nat[:, oo, :])
            wT = wTp.tile([P, VT, OUT2], BF16)  # [c_p, ct, o]
            for ct in range(VT):
                for half in range(OH):
                    ps = psA.tile([P, 512], BF16, tag="tpsA")
                    for j in range(4):
                        oo = half * 4 + j
                        nc.tensor.transpose(
                            ps[:, j * P:(j + 1) * P],
                            w_natb[:, oo, ct * P:(ct + 1) * P],
                            identity_bf[:],
                        )
                    nc.vector.tensor_copy(
                        wT[:, ct, half * 512:(half + 1) * 512], ps[:]
                    )

            # per-channel sum of squares in PSUM, accumulated over (n, vtile)
            s2_ps = [
                psS.tile([1, 512], F32, name=f"s2ps{i}", tag=f"s2ps{i}")
                for i in range(OH)
            ]

            for n in range(NS):
                xT = xTp.tile([P, VT, V], BF16)
                nc.sync.dma_start(
                    xT[:], xT_d.ap()[n].rearrange("(co p) v -> p co v", p=P)
                )
                for ct in range(VT):
                    # xsum[c] += sum_v xT[c, v]  (feeds S1 = xsum^T @ wT)
                    nc.vector.tensor_reduce(
                        xred[:], xT[:, ct, :], bass.mybir.AxisListType.X,
                        AluOpType.add,
                    )
                    nc.vector.tensor_add(
                        xsum[:, ct:ct + 1], xsum[:, ct:ct + 1], xred[:]
                    )

                # conv: hT(v, o) = xT^T @ wT, bf16 full-rate
                hTn = hTap.tile([P, VT, OUT2], BF16)
                slices = []
                for vt in range(VT):
                    for oh in range(OH):
                        ps = psC.tile([P, 512], F32, tag="cps")
                        for ct in range(VT):
                            nc.tensor.matmul(
                                ps[:],
                                xT[:, ct, vt * P:(vt + 1) * P],
                                wT[:, ct, oh * 512:(oh + 1) * 512],
                                start=(ct == 0),
                                stop=(ct == VT - 1),
                            )
                        dst = hTn[:, vt, oh * 512:(oh + 1) * 512]
                        if oh == 0:
                            nc.vector.tensor_copy(dst, ps[:])
                        else:
                            nc.scalar.copy(dst, ps[:])
                        sq = sqp.tile([P, 512], BF16, tag="sq")
                        nc.scalar.activation(sq[:], ps[:], AF.Square)
                        slices.append((vt, oh, sq))

                # S2 matmuls (after the convs so PE stays dense)
                for vt, oh, sq in slices:
                    nc.tensor.matmul(
                        s2_ps[oh][:], ones_col[:], sq[:],
                        start=(n == 0 and vt == 0),
                        stop=(n == NS - 1 and vt == VT - 1),
                    )
                nc.scalar.dma_start(hT_dram[n], hTn[:])

            # S1 = xsum^T @ wT via 8 tiny matmuls
            xsum_bf = rowsp.tile([P, VT], BF16)
            for ct in range(VT):
                nc.vector.tensor_copy(xsum_bf[:, ct:ct + 1], xsum[:, ct:ct + 1])
            s1_ps = [
                psS.tile([1, 512], F32, name=f"s1ps{i}", tag=f"s1ps{i}")
                for i in range(OH)
            ]
            for half in range(OH):
                for ct in range(VT):
                    nc.tensor.matmul(
                        s1_ps[half][:],
                        xsum_bf[:, ct:ct + 1],
                        wT[:, ct, half * 512:(half + 1) * 512],
                        start=(ct == 0),
                        stop=(ct == VT - 1),
                    )

            stats_loc = rowsp.tile([1, 4 * 512], F32)
            for i in range(OH):
                nc.vector.tensor_copy(
                    stats_loc[:, i * 512:(i + 1) * 512], s1_ps[i][:]
                )
                nc.vector.tensor_copy(
                    stats_loc[:, 1024 + i * 512:1024 + (i + 1) * 512], s2_ps[i][:]
                )

        st_in = dramp.tile([1, 4 * 512], F32)
        st_out = dramp.tile([NCORES, 4 * 512], F32)
        nc.scalar.dma_start(st_in[:], stats_loc[:])
        nc.gpsimd.collective_compute(
            "AllGather",
            AluOpType.bypass,
            replica_groups=[list(range(NCORES))],
            ins=[st_in.opt()],
            outs=[st_out.opt()],
        )

        # ------------- phase B pools (opened early so the A-path of the
        # first PREF batch elements overlaps the collective) -------------
        with (
            tc.tile_pool(name="Ak", bufs=2) as Akp,
            tc.tile_pool(name="Akb", bufs=PREF + 1) as Akbp,
            tc.tile_pool(name="Atf", bufs=1) as Atfp,
            tc.tile_pool(name="Atb", bufs=PREF + 1) as Atbp,
            tc.tile_pool(name="hTb", bufs=2) as hTbp,
            tc.tile_pool(name="xb", bufs=2) as xbp,
            tc.tile_pool(name="u", bufs=2) as up,
            tc.tile_pool(name="osb", bufs=2) as osbp,
            tc.tile_pool(name="psT", bufs=2, space="PSUM") as psT,
            tc.tile_pool(name="psU", bufs=3, space="PSUM") as psU,
            tc.tile_pool(name="psZ", bufs=3, space="PSUM") as psZ,
        ):
            akb_tiles = {}
            atb_tiles = {}

            def a_path(n):
                """A^T (exact fp32 output + bf16 copies) for one n; no
                dependency on the BN statistics, so it can hide the
                collective + row math."""
                Akb = Akbp.tile([P, K, VT, V], BF16, name="Akb", tag="Akb")
                Atb = Atbp.tile([P, K, VT, V], BF16, name="Atb", tag="Atb")
                akb_tiles[n] = Akb
                atb_tiles[n] = Atb
                for k in range(K):
                    Ak = Akp.tile([P, VT, V], F32, name="Ak", tag="Ak")
                    nc.sync.dma_start(
                        Ak[:],
                        A_d.ap()[n, k].rearrange("(mo p) e -> p mo e", p=P),
                    )
                    nc.scalar.copy(Akb[:, k], Ak[:])
                    Atf = Atfp.tile([P, VT, V], F32, name="Atf", tag="Atf")
                    for et in range(VT):
                        ps = psT.tile([P, 512], F32, tag="tps")
                        for mo in range(VT):
                            nc.tensor.transpose(
                                ps[:, mo * P:(mo + 1) * P],
                                Ak[:, mo, et * P:(et + 1) * P],
                                identity[:],
                            )
                        nc.scalar.copy(Atf[:, et, :], ps[:])
                        nc.scalar.copy(Atb[:, k, et, :], ps[:])
                    nc.scalar.dma_start(
                        At_d.ap()[n, k].rearrange("(eo p) m -> p eo m", p=P),
                        Atf[:],
                    )

            for n in range(PREF):
                a_path(n)

            # ---------------- BN rows + broadcasts ----------------
            st_all = hTbp.tile([NCORES, 4 * 512], F32, name="st_all", tag="hTn2")
            nc.sync.dma_start(st_all[:], st_out[:])
            stats_g = rowsp.tile([1, 4 * 512], F32)
            for i in range(4):
                ps = psT.tile([1, 512], F32, tag="tps")
                nc.tensor.matmul(
                    ps[:], ones8[:], st_all[:, i * 512:(i + 1) * 512],
                    start=True, stop=True,
                )
                nc.vector.tensor_copy(stats_g[:, i * 512:(i + 1) * 512], ps[:])
            mean_r = stats_g[:, :OUT2]
            nc.vector.tensor_scalar_mul(mean_r, mean_r, 1.0 / COUNT)
            ex2_r = stats_g[:, OUT2:]
            nc.vector.tensor_scalar_mul(ex2_r, ex2_r, 1.0 / COUNT)
            scr = stats_loc[:, :OUT2]
            nc.vector.tensor_mul(scr, mean_r, mean_r)
            nc.vector.tensor_sub(ex2_r, ex2_r, scr)   # ex2_r now = var
            nc.vector.tensor_scalar_add(ex2_r, ex2_r, BN_EPS)
            nc.scalar.activation(scr, ex2_r, AF.Sqrt)
            nc.vector.reciprocal(scr, scr)         # scr = 1/sqrt(var+eps)
            s_row = rowsp.tile([1, OUT2], F32R)
            nc.vector.tensor_mul(s_row[:], g_row[:], scr)
            t_row = rowsp.tile([1, OUT2], F32R)
            nc.vector.tensor_mul(t_row[:], mean_r, s_row[:])
            nc.vector.tensor_sub(t_row[:], be_row[:], t_row[:])

            # broadcast rows across partitions with rank-1 matmuls
            s_bc = constp.tile([P, OUT2], BF16)
            t_bc = constp.tile([P, OUT2], BF16)
            for row, bc in ((s_row, s_bc), (t_row, t_bc)):
                for half in range(OH):
                    ps = psT.tile([P, 512], F32, tag="tps")
                    nc.tensor.matmul(
                        ps[:], ones_row[:],
                        row[:, half * 512:(half + 1) * 512],
                        start=True, stop=True,
                    )
                    nc.vector.tensor_copy(bc[:, half * 512:(half + 1) * 512], ps[:])

            # (1 + eps_p) broadcast to a per-partition scalar column
            ep1 = rowsp.tile([1, 1], F32)
            nc.vector.tensor_scalar_add(ep1[:], ep_sb[:], 1.0)
            eps_bc = constp.tile([P, 1], F32)
            ps = psT.tile([P, 1], F32, tag="tps")
            nc.tensor.matmul(ps[:], ones_row_f[:], ep1[:], start=True, stop=True)
            nc.vector.tensor_copy(eps_bc[:], ps[:])

            # ---------------- phase B: BN apply + graph propagation -------
            for n in range(NS):
                if n >= PREF:
                    a_path(n)
                Akb = akb_tiles.pop(n)
                Atb = atb_tiles.pop(n)

                hTn = hTbp.tile([P, VT, OUT2], BF16, name="hTn2", tag="hTn2")
                nc.sync.dma_start(hTn[:], hT_dram[n])
                # BN apply in place (bf16)
                for mo in range(VT):
                    sl = hTn[:, mo, :]
                    nc.vector.tensor_mul(sl, sl, s_bc[:])
                    nc.vector.tensor_add(sl, sl, t_bc[:])

                xn2 = xbp.tile([P, VT, C], F32)
                nc.sync.dma_start(
                    xn2[:], x_d.ap()[n].rearrange("(vo p) c -> p vo c", p=P)
                )

                # u_k(e, c) = A_k^T @ hT_k   (bf16, full rate)
                u = up.tile([P, K, VT, 512], BF16)
                for k in range(K):
                    for et in range(VT):
                        ps = psU.tile([P, 512], F32, tag="ups")
                        for mo in range(VT):
                            nc.tensor.matmul(
                                ps[:],
                                Akb[:, k, mo, et * P:(et + 1) * P],
                                hTn[:, mo, k * 512:(k + 1) * 512],
                                start=(mo == 0),
                                stop=(mo == VT - 1),
                            )
                        nc.vector.tensor_copy(u[:, k, et, :], ps[:])

                # z(m, c) = sum_k A_k @ u_k ; out = (1+eps)*x + z
                out_sb = osbp.tile([P, VT, C], F32)
                for mt in range(VT):
                    ps = psZ.tile([P, 512], F32, tag="zps")
                    for k in range(K):
                        for et in range(VT):
                            nc.tensor.matmul(
                                ps[:],
                                Atb[:, k, et, mt * P:(mt + 1) * P],
                                u[:, k, et, :],
                                start=(k == 0 and et == 0),
                                stop=(k == K - 1 and et == VT - 1),
                            )
                    nc.vector.scalar_tensor_tensor(
                        out_sb[:, mt, :], xn2[:, mt, :], eps_bc[:], ps[:],
                        AluOpType.mult, AluOpType.add,
                    )
                nc.scalar.dma_start(
                    out_d.ap()[n].rearrange("(mo p) c -> p mo c", p=P),
                    out_sb[:],
                )


_NC_CACHE = {}


def _get_nc():
    if "nc" not in _NC_CACHE:
        nc = bacc.Bacc(
            "TRN2",
            target_bir_lowering=False,
            debug=False,
            enable_asserts=False,
            num_devices=NCORES,
        )
        build(nc)
        _NC_CACHE["nc"] = nc
    return _NC_CACHE["nc"]


def make_in_maps(x, A, conv_w, conv_b, gamma, beta, eps_p):
    import ml_dtypes
    ident = np.eye(P, dtype=np.float32)
    f32 = np.float32
    xTb = np.ascontiguousarray(
        np.transpose(np.asarray(x, dtype=f32), (0, 2, 1))
    ).astype(ml_dtypes.bfloat16)
    in_maps = []
    for c in range(NCORES):
        sl = slice(c * NS, (c + 1) * NS)
        in_maps.append({
            "x": np.ascontiguousarray(x[sl], dtype=f32),
            "xTb": xTb[sl],
            "A": np.ascontiguousarray(A[sl], dtype=f32),
            "conv_w": np.asarray(conv_w, dtype=f32),
            "conv_b": np.asarray(conv_b, dtype=f32),
            "gamma": np.asarray(gamma, dtype=f32),
            "beta": np.asarray(beta, dtype=f32),
            "eps_p": np.asarray(eps_p, dtype=f32),
            "identity": ident,
        })
    return in_maps


def run(inputs, trace=False):
    nc = _get_nc()
    in_maps = make_in_maps(**inputs)
    res = run_bass_kernel_spmd(nc, in_maps, list(range(NCORES)), trace=trace)
    out = np.concatenate([res.results[c]["out"] for c in range(NCORES)], axis=0)
    At = np.concatenate([res.results[c]["At"] for c in range(NCORES)], axis=0)
    return (out, At), res


def kernel(x, A, conv_w, conv_b, gamma, beta, eps_p):
    (out, At), _ = run(dict(
        x=np.asarray(x), A=np.asarray(A), conv_w=np.asarray(conv_w),
        conv_b=np.asarray(conv_b), gamma=np.asarray(gamma),
        beta=np.asarray(beta), eps_p=np.asarray(eps_p),
    ))
    return out, At


# revision 38
# speedup vs baseline: 1.2054x; 1.2054x over previous
"""Trainium2 Bass kernel for nn_ConvPersonGraphical (GNN message passing block).

Reference computation (per batch element n):
    h   = conv_w @ x[n]^T + conv_b          # 1x1 conv -> (2*OUT, V)
    h   = BN_train(h)                        # batch stats over (N, V) per channel
    hk  = h.reshape(K, OUT, V)
    t_k = h_k @ A[n,k]                       # (OUT, V) @ (V, V)
    y   = sum_k t_k @ A[n,k]^T               # (OUT, V)
    out = (1 + eps_p) * x[n] + y^T
    also returns At = transpose(A, (0,1,3,2))

Strategy: data-parallel over batch N across 8 NeuronCores (8 per core).
The conv and both propagation matmuls run at full TensorE rate in bf16
(numerically safe: the propagation output is tiny relative to the
residual x, which is carried in exact fp32).  BatchNorm batch stats:
S1 factors through the conv (S1 = colsum(x) @ w^T, almost free); S2 is
accumulated with ones-vector matmuls on the squared activations.  The
global reduction is one small AllReduce (preceded by a dummy warm-up
collective that absorbs the ncfw init cost under phase A).  A^T is
produced exactly on the TensorEngine in fp32 (needed as the stationary
operand of the second propagation matmul anyway) and streamed out as
the second output.  hT spills to DRAM between the phases to keep SBUF
free for deep pipelining.
"""

import os
import sys
import types

import numpy as np

import concourse.bass as bass
import concourse.bacc as bacc
import concourse.mybir as mybir
import concourse.tile as tile
from concourse.alu_op_type import AluOpType
from concourse.bass_utils import run_bass_kernel_spmd


def _install_ntff_hook_shim():
    """concourse.bass_utils reads antenv.axon_hooks.get_axon_ntff_profile_hook
    for trace=True under axon; this image's antenv lacks that module, so
    register an equivalent backed by the booted libaxon_pjrt.so."""
    try:
        import antenv.axon_hooks  # noqa: F401
        return
    except ImportError:
        pass
    try:
        import antenv
        from trn_agent_boot.trn_boot import _ntff_profile_via_ctypes

        hook = _ntff_profile_via_ctypes("/opt/axon/libaxon_pjrt.so")
        mod = types.ModuleType("antenv.axon_hooks")
        mod._hook = hook
        mod.get_axon_ntff_profile_hook = lambda: mod._hook

        def _set(h):
            mod._hook = h

        mod.set_axon_ntff_profile_hook = _set
        sys.modules["antenv.axon_hooks"] = mod
        antenv.axon_hooks = mod
    except Exception:
        pass


_install_ntff_hook_shim()

def _install_ldw_opt_patch():
    """Re-enable walrus LDWEIGHTS scheduling opt (hidden weight loads)."""
    if os.environ.get("KLDW", "0") != "1":
        return
    import concourse.bass_utils as _bu
    if getattr(_bu, "_ldw_patched", False):
        return
    _orig = _bu.run_command

    def _run(cmd, *a, **kw):
        if isinstance(cmd, list):
            cmd = ["--enable-ldw-opt=true" if c == "--enable-ldw-opt=false"
                   else c for c in cmd]
        return _orig(cmd, *a, **kw)

    _bu.run_command = _run
    _bu._ldw_patched = True


_install_ldw_opt_patch()

F32 = mybir.dt.float32
F32R = mybir.dt.float32r
BF16 = mybir.dt.bfloat16
AF = mybir.ActivationFunctionType

NCORES = 8
N, V, C = 64, 512, 512
K = 2
OUT2 = 1024            # 2 * OUT
NS = N // NCORES       # batch elements per core
COUNT = N * V          # BN statistic count (global, over all cores)
BN_EPS = 1e-5

P = 128                # SBUF partitions
VT = V // P            # 4 tiles of 128 along any 512 dim
OH = OUT2 // 512       # 2 halves of the channel dim (PSUM bank = 512 fp32)

PREF = 4               # batch elements whose A-path runs under the collective


def build(nc: bacc.Bacc):
    x_d = nc.declare_dram_parameter("x", [NS, V, C], F32, isOutput=False)
    xT_d = nc.declare_dram_parameter("xTb", [NS, C, V], BF16, isOutput=False)
    A_d = nc.declare_dram_parameter("A", [NS, K, V, V], F32, isOutput=False)
    w_d = nc.declare_dram_parameter("conv_w", [OUT2, C], F32, isOutput=False)
    nc.declare_dram_parameter("conv_b", [OUT2], F32, isOutput=False)
    g_d = nc.declare_dram_parameter("gamma", [OUT2], F32, isOutput=False)
    be_d = nc.declare_dram_parameter("beta", [OUT2], F32, isOutput=False)
    ep_d = nc.declare_dram_parameter("eps_p", [1], F32, isOutput=False)
    id_d = nc.declare_dram_parameter("identity", [P, P], F32, isOutput=False)
    out_d = nc.declare_dram_parameter("out", [NS, V, C], F32, isOutput=True)
    At_d = nc.declare_dram_parameter("At", [NS, K, V, V], F32, isOutput=True)

    with tile.TileContext(nc) as tc:
        _graph(nc, tc, x_d, xT_d, A_d, w_d, g_d, be_d, ep_d, id_d, out_d, At_d)
    nc.compile()
    return nc


def _graph(nc, tc, x_d, xT_d, A_d, w_d, g_d, be_d, ep_d, id_d, out_d, At_d):
    with (
        tc.tile_pool(name="const", bufs=1) as constp,
        tc.tile_pool(name="rows", bufs=1) as rowsp,
        tc.tile_pool(name="dram", bufs=1, space="DRAM") as dramp,
    ):
        # ---------------- constants ----------------
        identity = constp.tile([P, P], F32)
        nc.sync.dma_start(identity[:], id_d.ap())
        identity_bf = constp.tile([P, P], BF16)
        nc.vector.tensor_copy(identity_bf[:], identity[:])
        ones_col = constp.tile([P, 1], BF16)
        nc.vector.memset(ones_col[:], 1.0)
        ones_row_f = rowsp.tile([1, P], F32)
        nc.vector.memset(ones_row_f[:], 1.0)
        ones_row = rowsp.tile([1, P], F32R)
        nc.vector.tensor_copy(ones_row[:], ones_row_f[:])
        g_row = rowsp.tile([1, OUT2], F32)
        nc.sync.dma_start(g_row[:], g_d.ap().unsqueeze(0))
        be_row = rowsp.tile([1, OUT2], F32)
        nc.sync.dma_start(be_row[:], be_d.ap().unsqueeze(0))
        ep_sb = rowsp.tile([1, 1], F32)
        nc.sync.dma_start(ep_sb[:], ep_d.ap().unsqueeze(0))
        xsum = rowsp.tile([P, VT], F32)
        nc.vector.memset(xsum[:], 0.0)
        xred = rowsp.tile([P, 1], F32)

        # dummy collective issued first (no deps): absorbs ncfw init cost
        # under phase A so the real stats AllReduce hits the warm path
        dummy_sb = rowsp.tile([1, 32], F32)
        nc.vector.memset(dummy_sb[:], 0.0)
        dummy_in = dramp.tile([1, 32], F32)
        dummy_out = dramp.tile([NCORES, 32], F32)
        nc.sync.dma_start(dummy_in[:], dummy_sb[:])
        nc.gpsimd.collective_compute(
            "AllGather",
            AluOpType.bypass,
            replica_groups=[list(range(NCORES))],
            ins=[dummy_in.opt()],
            outs=[dummy_out.opt()],
        )
        ones8 = rowsp.tile([NCORES, 1], F32)
        nc.vector.memset(ones8[:], 1.0)

        # hT spill buffer in DRAM (bf16): [n][p][vt][o]
        hT_dram = dramp.tile([NS, P, VT, OUT2], BF16)

        # ---------------- phase A: conv + BN statistics ----------------
        with (
            tc.tile_pool(name="wload", bufs=1) as wl,
            tc.tile_pool(name="wT", bufs=1) as wTp,
            tc.tile_pool(name="xT", bufs=3) as xTp,
            tc.tile_pool(name="hTa", bufs=3) as hTap,
            tc.tile_pool(name="sq", bufs=4) as sqp,
            tc.tile_pool(name="psA", bufs=1, space="PSUM") as psA,   # transposes
            tc.tile_pool(name="psC", bufs=3, space="PSUM") as psC,   # conv
            tc.tile_pool(name="psS", bufs=1, space="PSUM") as psS,   # stats
        ):
            # transpose conv_w (OUT2, C) -> wT (C, OUT2), in bf16
            w_nat = wl.tile([P, OUT2 // P, C], F32)
            nc.sync.dma_start(
                w_nat[:], w_d.ap().rearrange("(oo p) c -> p oo c", p=P)
            )
            w_natb = wl.tile([P, OUT2 // P, C], BF16)
            for oo in range(OUT2 // P):
                nc.vector.tensor_copy(w_natb[:, oo, :], w_nat[:, oo, :])
            wT = wTp.tile([P, VT, OUT2], BF16)  # [c_p, ct, o]
            for ct in range(VT):
                for half in range(OH):
                    ps = psA.tile([P, 512], BF16, tag="tpsA")
                    for j in range(4):
                        oo = half * 4 + j
                        nc.tensor.transpose(
                            ps[:, j * P:(j + 1) * P],
                            w_natb[:, oo, ct * P:(ct + 1) * P],
                            identity_bf[:],
                        )
                    nc.vector.tensor_copy(
                        wT[:, ct, half * 512:(half + 1) * 512], ps[:]
                    )

            # per-channel sum of squares in PSUM, accumulated over (n, vtile)
            s2_ps = [
                psS.tile([1, 512], F32, name=f"s2ps{i}", tag=f"s2ps{i}")
                for i in range(OH)
            ]

            for n in range(NS):
                xT = xTp.tile([P, VT, V], BF16)
                nc.sync.dma_start(
                    xT[:], xT_d.ap()[n].rearrange("(co p) v -> p co v", p=P)
                )
                for ct in range(VT):
                    # xsum[c] += sum_v xT[c, v]  (feeds S1 = xsum^T @ wT)
                    nc.vector.tensor_reduce(
                        xred[:], xT[:, ct, :], bass.mybir.AxisListType.X,
                        AluOpType.add,
                    )
                    nc.vector.tensor_add(
                        xsum[:, ct:ct + 1], xsum[:, ct:ct + 1], xred[:]
                    )

                # conv: hT(v, o) = xT^T @ wT, bf16 full-rate
                hTn = hTap.tile([P, VT, OUT2], BF16)
                slices = []
                for vt in range(VT):
                    for oh in range(OH):
                        ps = psC.tile([P, 512], F32, tag="cps")
                        for ct in range(VT):
                            nc.tensor.matmul(
                                ps[:],
                                xT[:, ct, vt * P:(vt + 1) * P],
                                wT[:, ct, oh * 512:(oh + 1) * 512],
                                start=(ct == 0),
                                stop=(ct == VT - 1),
                            )
                        dst = hTn[:, vt, oh * 512:(oh + 1) * 512]
                        if oh == 0:
                            nc.vector.tensor_copy(dst, ps[:])
                        else:
                            nc.scalar.copy(dst, ps[:])
                        sq = sqp.tile([P, 512], BF16, tag="sq")
                        nc.scalar.activation(sq[:], ps[:], AF.Square)
                        slices.append((vt, oh, sq))

                # S2 matmuls (after the convs so PE stays dense)
                for vt, oh, sq in slices:
                    nc.tensor.matmul(
                        s2_ps[oh][:], ones_col[:], sq[:],
                        start=(n == 0 and vt == 0),
                        stop=(n == NS - 1 and vt == VT - 1),
                    )
                nc.sync.dma_start(hT_dram[n], hTn[:])

            # S1 = xsum^T @ wT via 8 tiny matmuls
            xsum_bf = rowsp.tile([P, VT], BF16)
            for ct in range(VT):
                nc.vector.tensor_copy(xsum_bf[:, ct:ct + 1], xsum[:, ct:ct + 1])
            s1_ps = [
                psS.tile([1, 512], F32, name=f"s1ps{i}", tag=f"s1ps{i}")
                for i in range(OH)
            ]
            for half in range(OH):
                for ct in range(VT):
                    nc.tensor.matmul(
                        s1_ps[half][:],
                        xsum_bf[:, ct:ct + 1],
                        wT[:, ct, half * 512:(half + 1) * 512],
                        start=(ct == 0),
                        stop=(ct == VT - 1),
                    )

            stats_loc = rowsp.tile([1, 4 * 512], F32)
            for i in range(OH):
                nc.vector.tensor_copy(
                    stats_loc[:, i * 512:(i + 1) * 512], s1_ps[i][:]
                )
                nc.vector.tensor_copy(
                    stats_loc[:, 1024 + i * 512:1024 + (i + 1) * 512], s2_ps[i][:]
                )

        st_in = dramp.tile([1, 4 * 512], F32)
        st_out = dramp.tile([NCORES, 4 * 512], F32)
        nc.sync.dma_start(st_in[:], stats_loc[:])
        nc.gpsimd.collective_compute(
            "AllGather",
            AluOpType.bypass,
            replica_groups=[list(range(NCORES))],
            ins=[st_in.opt()],
            outs=[st_out.opt()],
        )

        # ------------- phase B pools (opened early so the A-path of the
        # first PREF batch elements overlaps the collective) -------------
        with (
            tc.tile_pool(name="Ak", bufs=2) as Akp,
            tc.tile_pool(name="Akb", bufs=PREF + 1) as Akbp,
            tc.tile_pool(name="Atf", bufs=1) as Atfp,
            tc.tile_pool(name="Atb", bufs=PREF + 1) as Atbp,
            tc.tile_pool(name="hTb", bufs=2) as hTbp,
            tc.tile_pool(name="xb", bufs=2) as xbp,
            tc.tile_pool(name="u", bufs=2) as up,
            tc.tile_pool(name="osb", bufs=2) as osbp,
            tc.tile_pool(name="psT", bufs=2, space="PSUM") as psT,
            tc.tile_pool(name="psU", bufs=3, space="PSUM") as psU,
            tc.tile_pool(name="psZ", bufs=2, space="PSUM") as psZ,
            tc.tile_pool(name="psB", bufs=1, space="PSUM") as psB,
        ):
            akb_tiles = {}
            atb_tiles = {}

            def a_path(n):
                """A^T (exact fp32 output + bf16 copies) for one n; no
                dependency on the BN statistics, so it can hide the
                collective + row math."""
                Akb = Akbp.tile([P, K, VT, V], BF16, name="Akb", tag="Akb")
                Atb = Atbp.tile([P, K, VT, V], BF16, name="Atb", tag="Atb")
                akb_tiles[n] = Akb
                atb_tiles[n] = Atb
                for k in range(K):
                    Ak = Akp.tile([P, VT, V], F32, name="Ak", tag="Ak")
                    nc.sync.dma_start(
                        Ak[:],
                        A_d.ap()[n, k].rearrange("(mo p) e -> p mo e", p=P),
                    )
                    nc.scalar.copy(Akb[:, k], Ak[:])
                    Atf = Atfp.tile([P, VT, V], F32, name="Atf", tag="Atf")
                    for et in range(VT):
                        ps = psT.tile([P, 512], F32, tag="tps")
                        for mo in range(VT):
                            nc.tensor.transpose(
                                ps[:, mo * P:(mo + 1) * P],
                                Ak[:, mo, et * P:(et + 1) * P],
                                identity[:],
                            )
                        nc.scalar.copy(Atf[:, et, :], ps[:])
                        nc.scalar.copy(Atb[:, k, et, :], ps[:])
                    nc.sync.dma_start(
                        At_d.ap()[n, k].rearrange("(eo p) m -> p eo m", p=P),
                        Atf[:],
                    )

            for n in range(PREF):
                a_path(n)

            # ---------------- BN rows + broadcasts ----------------
            st_all = hTbp.tile([NCORES, 4 * 512], F32, name="st_all", tag="hTn2")
            nc.sync.dma_start(st_all[:], st_out[:])
            stats_g = rowsp.tile([1, 4 * 512], F32)
            for i in range(4):
                ps = psB.tile([1, 512], F32, tag="bcps")
                nc.tensor.matmul(
                    ps[:], ones8[:], st_all[:, i * 512:(i + 1) * 512],
                    start=True, stop=True,
                )
                nc.vector.tensor_copy(stats_g[:, i * 512:(i + 1) * 512], ps[:])
            mean_r = stats_g[:, :OUT2]
            nc.vector.tensor_scalar_mul(mean_r, mean_r, 1.0 / COUNT)
            ex2_r = stats_g[:, OUT2:]
            nc.vector.tensor_scalar_mul(ex2_r, ex2_r, 1.0 / COUNT)
            scr = stats_loc[:, :OUT2]
            nc.vector.tensor_mul(scr, mean_r, mean_r)
            nc.vector.tensor_sub(ex2_r, ex2_r, scr)   # ex2_r now = var
            nc.vector.tensor_scalar_add(ex2_r, ex2_r, BN_EPS)
            nc.scalar.activation(scr, ex2_r, AF.Sqrt)
            nc.vector.reciprocal(scr, scr)         # scr = 1/sqrt(var+eps)
            s_row = rowsp.tile([1, OUT2], F32R)
            nc.vector.tensor_mul(s_row[:], g_row[:], scr)
            t_row = rowsp.tile([1, OUT2], F32R)
            nc.vector.tensor_mul(t_row[:], mean_r, s_row[:])
            nc.vector.tensor_sub(t_row[:], be_row[:], t_row[:])

            # broadcast rows across partitions with rank-1 matmuls
            s_bc = constp.tile([P, OUT2], BF16)
            t_bc = constp.tile([P, OUT2], BF16)
            for row, bc in ((s_row, s_bc), (t_row, t_bc)):
                for half in range(OH):
                    ps = psB.tile([P, 512], F32, tag="bcps")
                    nc.tensor.matmul(
                        ps[:], ones_row[:],
                        row[:, half * 512:(half + 1) * 512],
                        start=True, stop=True,
                    )
                    nc.vector.tensor_copy(bc[:, half * 512:(half + 1) * 512], ps[:])

            # (1 + eps_p) broadcast to a per-partition scalar column
            ep1 = rowsp.tile([1, 1], F32)
            nc.vector.tensor_scalar_add(ep1[:], ep_sb[:], 1.0)
            eps_bc = constp.tile([P, 1], F32)
            ps = psB.tile([P, 1], F32, tag="bcps")
            nc.tensor.matmul(ps[:], ones_row_f[:], ep1[:], start=True, stop=True)
            nc.vector.tensor_copy(eps_bc[:], ps[:])

            # ---------------- phase B: BN apply + graph propagation -------
            for n in range(NS):
                if n >= PREF:
                    a_path(n)
                Akb = akb_tiles.pop(n)
                Atb = atb_tiles.pop(n)

                hTn = hTbp.tile([P, VT, OUT2], BF16, name="hTn2", tag="hTn2")
                nc.sync.dma_start(hTn[:], hT_dram[n])
                # BN apply in place (bf16)
                for mo in range(VT):
                    sl = hTn[:, mo, :]
                    nc.vector.tensor_mul(sl, sl, s_bc[:])
                    nc.vector.tensor_add(sl, sl, t_bc[:])

                xn2 = xbp.tile([P, VT, C], F32)
                nc.sync.dma_start(
                    xn2[:], x_d.ap()[n].rearrange("(vo p) c -> p vo c", p=P)
                )

                # u_k(e, c) = A_k^T @ hT_k   (bf16, full rate)
                u = up.tile([P, K, VT, 512], BF16)
                for k in range(K):
                    for et in range(VT):
                        ps = psU.tile([P, 512], F32, tag="ups")
                        for mo in range(VT):
                            nc.tensor.matmul(
                                ps[:],
                                Akb[:, k, mo, et * P:(et + 1) * P],
                                hTn[:, mo, k * 512:(k + 1) * 512],
                                start=(mo == 0),
                                stop=(mo == VT - 1),
                            )
                        nc.vector.tensor_copy(u[:, k, et, :], ps[:])

                # z(m, c) = sum_k A_k @ u_k ; out = (1+eps)*x + z
                out_sb = osbp.tile([P, VT, C], F32)
                for mt in range(VT):
                    ps = psZ.tile([P, 512], F32, tag="zps")
                    for k in range(K):
                        for et in range(VT):
                            nc.tensor.matmul(
                                ps[:],
                                Atb[:, k, et, mt * P:(mt + 1) * P],
                                u[:, k, et, :],
                                start=(k == 0 and et == 0),
                                stop=(k == K - 1 and et == VT - 1),
                            )
                    nc.vector.scalar_tensor_tensor(
                        out_sb[:, mt, :], xn2[:, mt, :], eps_bc[:], ps[:],
                        AluOpType.mult, AluOpType.add,
                    )
                nc.sync.dma_start(
                    out_d.ap()[n].rearrange("(mo p) c -> p mo c", p=P),
                    out_sb[:],
                )


_NC_CACHE = {}


def _get_nc():
    if "nc" not in _NC_CACHE:
        nc = bacc.Bacc(
            "TRN2",
            target_bir_lowering=False,
            debug=False,
            enable_asserts=False,
            num_devices=NCORES,
        )
        build(nc)
        _NC_CACHE["nc"] = nc
    return _NC_CACHE["nc"]


def make_in_maps(x, A, conv_w, conv_b, gamma, beta, eps_p):
    import ml_dtypes
    ident = np.eye(P, dtype=np.float32)
    f32 = np.float32
    xTb = np.ascontiguousarray(
        np.transpose(np.asarray(x, dtype=f32), (0, 2, 1))
    ).astype(ml_dtypes.bfloat16)
    in_maps = []
    for c in range(NCORES):
        sl = slice(c * NS, (c + 1) * NS)
        in_maps.append({
            "x": np.ascontiguousarray(x[sl], dtype=f32),
            "xTb": xTb[sl],
            "A": np.ascontiguousarray(A[sl], dtype=f32),
            "conv_w": np.asarray(conv_w, dtype=f32),
            "conv_b": np.asarray(conv_b, dtype=f32),
            "gamma": np.asarray(gamma, dtype=f32),
            "beta": np.asarray(beta, dtype=f32),
            "eps_p": np.asarray(eps_p, dtype=f32),
            "identity": ident,
        })
    return in_maps


def run(inputs, trace=False):
    nc = _get_nc()
    in_maps = make_in_maps(**inputs)
    res = run_bass_kernel_spmd(nc, in_maps, list(range(NCORES)), trace=trace)
    out = np.concatenate([res.results[c]["out"] for c in range(NCORES)], axis=0)
    At = np.concatenate([res.results[c]["At"] for c in range(NCORES)], axis=0)
    return (out, At), res


def kernel(x, A, conv_w, conv_b, gamma, beta, eps_p):
    (out, At), _ = run(dict(
        x=np.asarray(x), A=np.asarray(A), conv_w=np.asarray(conv_w),
        conv_b=np.asarray(conv_b), gamma=np.asarray(gamma),
        beta=np.asarray(beta), eps_p=np.asarray(eps_p),
    ))
    return out, At
